# revision 1
# baseline (speedup 1.0000x reference)
"""nn_PointHead kernel for 8 Trainium2 NeuronCores.

kernel(**inputs) takes the FULL inputs (x [16,512,60,80] + conv weights),
shards the batch 2 images/core across 8 cores (pure data parallel), runs the
Bass program (conv stack in fp16 3-pass split matmuls + fp32 conv3/softmax +
exact fp32 simple_nms), and returns the full [16,480,640] fp32 output.

Self-contained: only needs the concourse runtime at /opt/trn_rl_repo.
"""
import os
import sys

sys.path.insert(0, '/opt/trn_rl_repo')

from contextlib import ExitStack

import numpy as np

import concourse.bass as bass
import concourse.bacc as bacc
import concourse.tile as tile
import concourse.mybir as mybir
from concourse.bass_utils import run_bass_kernel_spmd

F32 = mybir.dt.float32
F16 = mybir.dt.float16
BF16 = mybir.dt.bfloat16
MAX = mybir.AluOpType.max
EQ = mybir.AluOpType.is_equal
MULT = mybir.AluOpType.mult
SUB = mybir.AluOpType.subtract
Act = mybir.ActivationFunctionType

PAD_NEG = -1.0e30
Hc, Wc = 60, 80
H, W = 480, 640
N_CORES = 8
IMGS = 2  # images per core


def _split16(a):
    hi = a.astype(np.float16)
    lo = (a.astype(np.float32) - hi.astype(np.float32)).astype(np.float16)
    return hi, lo


def _prep_weights(wPa, bPa, wPd, bPd, wPg, bPg):
    waT = np.ascontiguousarray(
        wPa.reshape(2, 128, 4, 128, 3, 3).transpose(2, 4, 5, 0, 3, 1))
    wa_hi, wa_lo = _split16(waT)
    wdT = np.ascontiguousarray(
        wPd.reshape(1, 128, 2, 128, 3, 3).transpose(2, 4, 5, 0, 3, 1))
    wd_hi, wd_lo = _split16(wdT)
    wgT = np.ascontiguousarray(wPg[:, :, 0, 0].T).astype(np.float32)
    return {
        'wa_hi': wa_hi, 'wa_lo': wa_lo, 'wd_hi': wd_hi, 'wd_lo': wd_lo,
        'wgT': wgT,
        'ba': np.ascontiguousarray(bPa.reshape(2, 128)).astype(np.float32),
        'bd': np.ascontiguousarray(bPd.reshape(1, 128)).astype(np.float32),
        'bg': np.ascontiguousarray(bPg.reshape(1, 65)).astype(np.float32),
    }


class _P:
    pass


def _declare_io(nc, imgs):
    p = _P()
    p.imgs = imgs
    p.x_hi = nc.dram_tensor('x_hi', (imgs, 512, Hc, Wc), F16, kind='ExternalInput').ap()
    p.x_lo = nc.dram_tensor('x_lo', (imgs, 512, Hc, Wc), F16, kind='ExternalInput').ap()
    p.wa_hi = nc.dram_tensor('wa_hi', (4, 3, 3, 2, 128, 128), F16, kind='ExternalInput').ap()
    p.wa_lo = nc.dram_tensor('wa_lo', (4, 3, 3, 2, 128, 128), F16, kind='ExternalInput').ap()
    p.wd_hi = nc.dram_tensor('wd_hi', (2, 3, 3, 1, 128, 128), F16, kind='ExternalInput').ap()
    p.wd_lo = nc.dram_tensor('wd_lo', (2, 3, 3, 1, 128, 128), F16, kind='ExternalInput').ap()
    p.wgT = nc.dram_tensor('wgT', (128, 65), F32, kind='ExternalInput').ap()
    p.ba = nc.dram_tensor('ba', (2, 128), F32, kind='ExternalInput').ap()
    p.bd = nc.dram_tensor('bd', (1, 128), F32, kind='ExternalInput').ap()
    p.bg = nc.dram_tensor('bg', (1, 65), F32, kind='ExternalInput').ap()
    p.y = nc.dram_tensor('y', (imgs, H, W), F32, kind='ExternalOutput').ap()
    p.full = nc.dram_tensor('full', (imgs, H, W), F32, kind='Internal').ap()
    return p


def _carve(t, dtype, col0, ncols, parts=124):
    v = t.bitcast(dtype) if t.dtype != dtype else t
    return v[0:parts, col0: col0 + ncols]


def _emit_mp9(nc, T, MP, tmpA, tmpB, slabA, slabB, slab3A, slab3B):
    """Centered 9x9 max-pool of T -> MP (valid [:, :, 8:648]); see kernel_lib."""
    TT = nc.vector.tensor_tensor
    X = slice(8, 648)
    TT(tmpA[:, :, 4:652], T[:, :, 4:652], T[:, :, 5:653], op=MAX)
    TT(tmpA[:, :, 4:652], tmpA[:, :, 4:652], T[:, :, 3:651], op=MAX)
    TT(tmpB[:, :, X], tmpA[:, :, X], tmpA[:, :, 11:651], op=MAX)
    TT(tmpB[:, :, X], tmpB[:, :, X], tmpA[:, :, 5:645], op=MAX)
    nc.sync.dma_start(slabA[0:123, X], tmpB[1:124, 0, X])
    nc.sync.dma_start(slabB[1:124, X], tmpB[0:123, 3, X])
    TT(tmpA[:, 0:3, X], tmpB[:, 0:3, X], tmpB[:, 1:4, X], op=MAX)
    TT(tmpA[:, 3, X], tmpB[:, 3, X], slabA[:, X], op=MAX)
    TT(tmpA[:, 1:4, X], tmpA[:, 1:4, X], tmpB[:, 0:3, X], op=MAX)
    TT(tmpA[:, 0, X], tmpA[:, 0, X], slabB[:, X], op=MAX)
    nc.sync.dma_start(slab3A[0:123, :, X], tmpA[1:124, 0:3, X])
    nc.sync.dma_start(slab3B[1:124, :, X], tmpA[0:123, 1:4, X])
    TT(MP[:, 0, X], tmpA[:, 0, X], tmpA[:, 3, X], op=MAX)
    TT(MP[:, 1:4, X], tmpA[:, 1:4, X], slab3A[:, :, X], op=MAX)
    TT(MP[:, 3, X], MP[:, 3, X], tmpA[:, 0, X], op=MAX)
    TT(MP[:, 0:3, X], MP[:, 0:3, X], slab3B[:, :, X], op=MAX)


def _build(ctx, tc, p):
    nc = tc.nc
    V = nc.vector
    TT = V.tensor_tensor

    wpool = ctx.enter_context(tc.tile_pool(name='weights', bufs=1))
    cpool = ctx.enter_context(tc.tile_pool(name='conv', bufs=1))
    spool = ctx.enter_context(tc.tile_pool(name='small', bufs=1))
    hpool = ctx.enter_context(tc.tile_pool(name='chunks', bufs=4))
    lpool = ctx.enter_context(tc.tile_pool(name='slabs', bufs=1))
    pps = ctx.enter_context(tc.tile_pool(name='convps', bufs=6, space='PSUM'))
    pps3 = ctx.enter_context(tc.tile_pool(name='ps3', bufs=2, space='PSUM'))

    wa, wd = [], []
    for lst, nm, d, n in ((wa, 'wa_hi', p.wa_hi, 72), (wa, 'wa_lo', p.wa_lo, 72),
                          (wd, 'wd_hi', p.wd_hi, 18), (wd, 'wd_lo', p.wd_lo, 18)):
        t = wpool.tile([128, n, 128], F16, tag=nm, name=nm)
        nc.sync.dma_start(t, d.rearrange('kt dy dx mt k m -> k (kt dy dx mt) m'))
        lst.append(t)
    wg = wpool.tile([128, 65], F32, tag='wg')
    nc.sync.dma_start(wg, p.wgT)
    ba = wpool.tile([128, 2], F32, tag='ba')
    nc.sync.dma_start(ba, p.ba.rearrange('mt k -> k mt'))
    bd = wpool.tile([128, 1], F32, tag='bd')
    nc.sync.dma_start(bd, p.bd.rearrange('mt k -> k mt'))
    bg = wpool.tile([1, 65], F32, tag='bg')
    nc.sync.dma_start(bg, p.bg)
    ones1 = wpool.tile([1, 80], F32, tag='ones1')
    V.memset(ones1, 1.0)

    slabA = lpool.tile([124, 656], F32, tag='slabA')
    slabB = lpool.tile([124, 656], F32, tag='slabB')
    slab3A = lpool.tile([124, 3, 656], F32, tag='slab3A')
    slab3B = lpool.tile([124, 3, 656], F32, tag='slab3B')
    for s in (slabA, slabB, slab3A, slab3B):
        nc.gpsimd.memset(s.bitcast(BF16), PAD_NEG)
    slabA_b = slabA.bitcast(BF16)[:, 0:656]
    slabB_b = slabB.bitcast(BF16)[:, 0:656]
    slab3A_b = slab3A.bitcast(BF16)[:, :, 0:656]
    slab3B_b = slab3B.bitcast(BF16)[:, :, 0:656]

    VAL = (slice(0, 124), slice(0, 4), slice(8, 648))

    for img in range(p.imgs):
        # conv1 — x streamed in two half-height slabs
        xpA = cpool.tile([128, 4 * 32 * 82], F16, tag='xpA')
        xpB = cpool.tile([128, 4 * 32 * 82], F16, tag='xpB')
        h1A = cpool.tile([128, 2 * 62 * 82], F16, tag='h1A')
        h1B = cpool.tile([128, 2 * 62 * 82], F16, tag='h1B')
        h2 = cpool.tile([128, 60 * 80], F32, tag='h2')
        h1Av = h1A.rearrange('p (mt h w) -> p mt h w', mt=2, h=62)
        h1Bv = h1B.rearrange('p (mt h w) -> p mt h w', mt=2, h=62)
        for hp in (h1Av, h1Bv):
            nc.gpsimd.memset(hp[:, :, 0, :], 0.0)
            nc.gpsimd.memset(hp[:, :, 61, :], 0.0)
            nc.gpsimd.memset(hp[:, :, 1:61, 0], 0.0)
            nc.gpsimd.memset(hp[:, :, 1:61, 81], 0.0)

        def conv3x3(x_pair, w_pair, nkt, nmt, mt, ps_list, y0s):
            n = len(ps_list)
            first = [True] * n
            for kt in range(nkt):
                for dy in range(3):
                    for dx in range(3):
                        widx = (kt * 9 + dy * 3 + dx) * nmt + mt
                        for wt, xt in ((0, 0), (0, 1), (1, 0)):
                            w_tile = w_pair[wt][:, widx, :]
                            last = (kt == nkt - 1 and dy == 2 and dx == 2
                                    and (wt, xt) == (1, 0))
                            for c in range(n):
                                rhs = x_pair[xt][:, kt, y0s[c] + dy: y0s[c] + dy + 6,
                                                 dx: dx + 80]
                                nc.tensor.matmul(ps_list[c], w_tile, rhs,
                                                 start=first[c], stop=last)
                                first[c] = False

        for half in (0, 1):
            xpAv = xpA.rearrange('p (kt h w) -> p kt h w', kt=4, h=32)
            xpBv = xpB.rearrange('p (kt h w) -> p kt h w', kt=4, h=32)
            for xv in (xpAv, xpBv):
                nc.gpsimd.memset(xv[:, :, 0 if half == 0 else 31, :], 0.0)
                nc.gpsimd.memset(xv[:, :, :, 0], 0.0)
                nc.gpsimd.memset(xv[:, :, :, 81], 0.0)
            lr = slice(1, 32) if half == 0 else slice(0, 31)
            xr = slice(0, 31) if half == 0 else slice(29, 60)
            for xv, src in ((xpAv, p.x_hi), (xpBv, p.x_lo)):
                for kt in range(4):
                    nc.sync.dma_start(
                        xv[:, kt, lr, 1:81],
                        src[img, 128 * kt: 128 * (kt + 1), xr, :])
            for mt in range(2):
                ps_list = [pps.tile([128, 480], F32, tag='cps',
                                    name=f'cps{img}_{half}_{mt}_{c}')
                           for c in range(5)]
                conv3x3((xpAv, xpBv), wa, 4, 2, mt, ps_list,
                        [6 * c for c in range(5)])
                for c in range(5):
                    ch = 5 * half + c
                    h1f = hpool.tile([128, 6, 80], F32, tag='h1f')
                    nc.scalar.activation(h1f, ps_list[c].rearrange(
                        'p (a b) -> p a b', a=6), Act.Relu, bias=ba[:, mt:mt + 1])
                    reg_hi = h1Av[:, mt, 6 * ch + 1: 6 * ch + 7, 1:81]
                    reg_lo = h1Bv[:, mt, 6 * ch + 1: 6 * ch + 7, 1:81]
                    nc.scalar.copy(reg_hi, h1f)
                    TT(reg_lo, h1f, reg_hi, op=SUB)

        # conv2
        h2v = h2.rearrange('p (h w) -> p h w', h=60)
        for start, n in ((0, 6), (6, 4)):
            ps_list = [pps.tile([128, 480], F32, tag='cps',
                                name=f'c2ps{img}_{start}_{c}') for c in range(n)]
            conv3x3((h1Av, h1Bv), wd, 2, 1, 0, ps_list,
                    [6 * (start + c) for c in range(n)])
            for c in range(n):
                ch = start + c
                nc.scalar.activation(
                    h2v[:, 6 * ch: 6 * ch + 6, :],
                    ps_list[c].rearrange('p (a b) -> p a b', a=6),
                    Act.Relu, bias=bd)

        # conv3 + softmax
        exp_all = _carve(xpA, F32, 0, 3900, parts=80).rearrange(
            'p (t c) -> p t c', t=60)
        sums = spool.tile([80, 60], F32, tag='sums')
        rinv = spool.tile([80, 60], F32, tag='rinv')
        for t in range(60):
            ps3 = pps3.tile([80, 65], F32, tag='ps3', name=f'ps3_{img}_{t}')
            nc.tensor.matmul(ps3, h2[:, 80 * t: 80 * (t + 1)], wg,
                             start=True, stop=False)
            nc.tensor.matmul(ps3, ones1[0:1, :], bg[0:1, :], start=False, stop=True)
            nc.scalar.activation(exp_all[:, t, :], ps3, Act.Exp,
                                 accum_out=sums[:, t:t + 1])
        V.reciprocal(rinv, sums)
        scores = _carve(h2, F32, 0, 3840, parts=80).rearrange(
            'p (t c) -> p t c', t=60)
        TT(scores, exp_all[:, :, 0:64],
           rinv[:, :, None].to_broadcast((80, 60, 64)), op=MULT)
        sc4 = scores.rearrange('p t (r cc) -> p t r cc', r=8)
        for g in range(6):
            nc.sync.dma_start(
                bass.AP(tensor=p.full.tensor,
                        offset=p.full.offset + img * H * W + 5120 * 10 * g,
                        ap=[[8, 80], [5120, 10], [640, 8], [1, 8]]),
                sc4[:, 10 * g: 10 * (g + 1)])

        # NMS
        b4 = lambda v: v.rearrange('p (b x) -> p b x', b=4)
        s_t = b4(_carve(xpA, F32, 0, 2624))
        tmpA = b4(_carve(xpA, F32, 2624, 2624))
        tmpB = b4(_carve(xpB, F32, 0, 2624))
        mp_t = b4(_carve(xpB, F32, 2624, 2624))
        supp_t = b4(_carve(h1A, F32, 0, 2624))
        dil_t = b4(_carve(h1A, BF16, 5248, 2624))
        m_t = b4(_carve(h1B, BF16, 0, 2624))
        tmpAb = b4(_carve(h1B, BF16, 2624, 2624))
        tmpBb = b4(_carve(h1B, BF16, 5248, 2624))
        for t in (s_t, tmpA, tmpB, mp_t, supp_t, dil_t, m_t, tmpAb, tmpBb):
            nc.gpsimd.memset(t, PAD_NEG)

        nc.sync.dma_start(
            s_t[2:122, :, 8:648],
            bass.AP(tensor=p.full.tensor, offset=p.full.offset + img * H * W,
                    ap=[[4 * W, 120], [W, 4], [1, W]]))

        _emit_mp9(nc, s_t, mp_t, tmpA, tmpB, slabA, slabB, slab3A, slab3B)
        TT(m_t[VAL], s_t[VAL], mp_t[VAL], op=EQ)
        for _ in range(2):
            _emit_mp9(nc, m_t, dil_t, tmpAb, tmpBb, slabA_b, slabB_b,
                      slab3A_b, slab3B_b)
            V.scalar_tensor_tensor(supp_t[VAL], dil_t[VAL], 0.0, s_t[VAL],
                                   op0=EQ, op1=MULT)
            _emit_mp9(nc, supp_t, mp_t, tmpA, tmpB, slabA, slabB, slab3A, slab3B)
            TT(tmpAb[VAL], supp_t[VAL], mp_t[VAL], op=EQ)
            V.scalar_tensor_tensor(tmpBb[VAL], dil_t[VAL], 0.0, tmpAb[VAL],
                                   op0=EQ, op1=MULT)
            TT(m_t[VAL], m_t[VAL], tmpBb[VAL], op=MAX)
        TT(tmpA[VAL], m_t[VAL], s_t[VAL], op=MULT)
        nc.sync.dma_start(p.y[img].rearrange('(pp b) x -> pp b x', b=4),
                          tmpA[2:122, :, 8:648])


_CACHE = {}
LAST_RESULTS = None


def _get_program():
    if 'prog' not in _CACHE:
        nc = bacc.Bacc('TRN2', target_bir_lowering=False, debug=False)
        p = _declare_io(nc, IMGS)
        with tile.TileContext(nc) as tc, ExitStack() as ctx:
            _build(ctx, tc, p)
        nc.compile()
        _CACHE['prog'] = (nc, p)
    return _CACHE['prog']


def kernel(x, wPa, bPa, wPd, bPd, wPg, bPg):
    global LAST_RESULTS
    nc, p = _get_program()
    x = np.asarray(x, dtype=np.float32)
    wmap = _prep_weights(np.asarray(wPa), np.asarray(bPa), np.asarray(wPd),
                         np.asarray(bPd), np.asarray(wPg), np.asarray(bPg))
    x_hi, x_lo = _split16(x)
    in_maps = []
    for c in range(N_CORES):
        m = dict(wmap)
        m['x_hi'] = np.ascontiguousarray(x_hi[IMGS * c: IMGS * (c + 1)])
        m['x_lo'] = np.ascontiguousarray(x_lo[IMGS * c: IMGS * (c + 1)])
        in_maps.append(m)
    trace = bool(os.environ.get('KERNEL_TRACE'))
    res = run_bass_kernel_spmd(
        nc, in_maps, list(range(N_CORES)), trace=trace,
        tmpdir=os.environ.get('KERNEL_TRACE_DIR') or None)
    LAST_RESULTS = res
    return np.concatenate([res.results[c]['y'] for c in range(N_CORES)], axis=0)
